# revision 2
# baseline (speedup 1.0000x reference)
"""Trainium2 Bass kernel for nn_InvertibleFourierGaussianFilter.

The reference "Fourier Gaussian filter" (FWHM=1.0mm, spacing 1.0) is
mathematically a 5x5 separable Gaussian convolution (sigma ~ 0.4247 px,
taps at -2..2): reflect-padded by 2 rows (Y), circular by 2 cols (X).
The rfft2/irfft2 round trip in the reference is just its implementation.

Strategy: pure data parallel over the batch (16 views per core x 8
cores).  Host pads each view to (772, 1028) (reflect rows / wrap cols)
so the device kernel is a pure "valid" 5x5 separable stencil:
  - Y pass: banded-matrix matmul on the tensor engine (PSUM out)
  - X pass: one scaled copy on the scalar engine + 4 scalar_tensor_tensor
    FMAs on the vector engine.
"""

import sys

import numpy as np

sys.path.insert(0, "/opt/trn_rl_repo")

import concourse.bacc as bacc
import concourse.mybir as mybir
import concourse.tile as tile
from concourse.bass_utils import run_bass_kernel_spmd

N_CORES = 8
B_FULL, H, W = 128, 768, 1024
B_LOC = B_FULL // N_CORES  # 16 views per core
PAD = 2
HP, WP = H + 2 * PAD, W + 2 * PAD  # 772, 1028
CHUNK = 124  # output rows per full chunk (128 input rows incl. halo)


def _taps() -> np.ndarray:
    """Normalized 1-D Gaussian taps, identical (up to f32 rounding) to the
    factorization of the reference's normalized 5x5 kernel."""
    sigma = 1.0 / 2.35482
    d = np.arange(-PAD, PAD + 1, dtype=np.float64)
    w = np.exp(-(d * d) / (2.0 * sigma * sigma))
    return (w / w.sum()).astype(np.float32)


def _banded(taps: np.ndarray) -> np.ndarray:
    """B[pi, po] = taps[pi - po]: matmul(lhsT=B[:cin,:cout], rhs=x) gives
    t[po, :] = sum_d taps[d] * x[po + d, :] (valid Y correlation)."""
    Bm = np.zeros((128, CHUNK), np.float32)
    for po in range(CHUNK):
        Bm[po : po + 2 * PAD + 1, po] = taps
    return Bm


def _row_chunks():
    """(r0, cin, cout) covering all 768 output rows of one padded view."""
    chunks = []
    r0 = 0
    while r0 < H:
        cout = min(CHUNK, H - r0)
        chunks.append((r0, cout + 2 * PAD, cout))
        r0 += cout
    return chunks


def _build_program():
    f32 = mybir.dt.float32
    nc = bacc.Bacc("TRN2", target_bir_lowering=False, debug=False)
    xp = nc.dram_tensor("xp", [B_LOC, HP, WP], f32, kind="ExternalInput")
    bY = nc.dram_tensor("bY", [128, CHUNK], f32, kind="ExternalInput")
    y = nc.dram_tensor("y", [B_LOC, H, W], f32, kind="ExternalOutput")
    wx = _taps()

    with tile.TileContext(nc) as tc:
        with (
            tc.tile_pool(name="const", bufs=1) as cpool,
            tc.tile_pool(name="xin", bufs=4) as inpool,
            tc.tile_pool(name="ps", bufs=2, space="PSUM") as pspool,
            tc.tile_pool(name="xout", bufs=4) as outpool,
        ):
            bt = cpool.tile([128, CHUNK], f32)
            nc.sync.dma_start(bt[:], bY[:])
            for img in range(B_LOC):
                for r0, cin, cout in _row_chunks():
                    xin = inpool.tile([128, WP], f32, tag="xin")
                    nc.sync.dma_start(xin[:cin, :], xp[img, r0 : r0 + cin, :])
                    t = pspool.tile([CHUNK, WP], f32, tag="ps")
                    for c0 in range(0, WP, 512):
                        c1 = min(c0 + 512, WP)
                        nc.tensor.matmul(
                            t[:cout, c0:c1],
                            bt[:cin, :cout],
                            xin[:cin, c0:c1],
                            start=True,
                            stop=True,
                        )
                    out = outpool.tile([CHUNK, W], f32, tag="xout")
                    nc.scalar.activation(
                        out[:cout, :],
                        t[:cout, PAD : PAD + W],
                        mybir.ActivationFunctionType.Copy,
                        scale=float(wx[PAD]),
                    )
                    for d in (0, 1, 3, 4):
                        nc.vector.scalar_tensor_tensor(
                            out[:cout, :],
                            t[:cout, d : d + W],
                            float(wx[d]),
                            out[:cout, :],
                            op0=mybir.AluOpType.mult,
                            op1=mybir.AluOpType.add,
                        )
                    nc.sync.dma_start(y[img, r0 : r0 + cout, :], out[:cout, :])
    nc.finalize()
    return nc


_CACHE: dict = {}


def _get_program():
    if "nc" not in _CACHE:
        _CACHE["nc"] = _build_program()
    return _CACHE["nc"]


def _run(x, trace: bool = False, **spmd_kwargs):
    x = np.ascontiguousarray(np.asarray(x, dtype=np.float32))
    assert x.shape == (B_FULL, H, W), x.shape
    xq = np.pad(x, ((0, 0), (PAD, PAD), (0, 0)), mode="reflect")
    xq = np.pad(xq, ((0, 0), (0, 0), (PAD, PAD)), mode="wrap")
    Bm = _banded(_taps())
    in_maps = [
        {"xp": np.ascontiguousarray(xq[i * B_LOC : (i + 1) * B_LOC]), "bY": Bm}
        for i in range(N_CORES)
    ]
    nc = _get_program()
    res = run_bass_kernel_spmd(nc, in_maps, list(range(N_CORES)), trace=trace, **spmd_kwargs)
    out = np.concatenate([r["y"] for r in res.results], axis=0)
    return np.ascontiguousarray(out.astype(np.float32, copy=False)), res


def kernel(x):
    out, _ = _run(x)
    return out


# revision 5
# speedup vs baseline: 1.1184x; 1.1184x over previous
"""Trainium2 Bass kernel for nn_InvertibleFourierGaussianFilter.

The reference "Fourier Gaussian filter" (FWHM=1.0mm, spacing 1.0) is
mathematically a 5x5 separable Gaussian convolution (sigma ~ 0.4247 px,
taps at -2..2): reflect-padded by 2 rows (Y), circular by 2 cols (X).
The rfft2/irfft2 round trip in the reference is just its implementation.

Strategy: pure data parallel over the batch (16 views per core x 8
cores).  Host pads each view (reflect rows / wrap cols) so the device
kernel is a pure "valid" separable stencil.  Per 124-row chunk:

  - Y pass (all 5 taps) + the tiny X +-2 taps (coeff 1.35e-5) in one
    PSUM accumulation on the tensor engine: one fp32 banded matmul
    (exact) + one bf16 banded matmul whose operand x[c]+x[c+4] is
    pre-summed on the otherwise-idle gpsimd engine.
  - X center tap: scaled copy on the scalar engine (exact fp32).
  - X +-1 taps: tensor_tensor add + scalar_tensor_tensor FMA on the
    vector engine (exact fp32).

Total error vs the fp32 FFT reference ~2e-6 (bf16 on the 1.35e-5-weight
taps contributes ~1e-7; a ~1e-6 term comes from those taps also being
picked up, doubly attenuated, by the +-1 tap reads).
"""

import sys

import numpy as np

sys.path.insert(0, "/opt/trn_rl_repo")

import ml_dtypes
import concourse.bacc as bacc
import concourse.mybir as mybir
import concourse.tile as tile
from concourse.bass_utils import run_bass_kernel_spmd

N_CORES = 8
B_FULL, H, W = 128, 768, 1024
B_LOC = B_FULL // N_CORES  # 16 views per core
PAD = 2  # stencil radius
PADX = 4  # host wrap-padding per side along X (extra 2 for the +-2-tap reads)
HP, WP = H + 2 * PAD, W + 2 * PADX  # 772, 1032
WT = W + 2 * PAD  # 1028: width of the Y-pass intermediate t
CHUNK = 124  # output rows per full chunk (128 input rows incl. halo)

MODE = "v2"  # "v1" | "d" (drop X +-2 taps) | "v2" (bf16 X +-2 taps)


def _taps() -> np.ndarray:
    """Normalized 1-D Gaussian taps, identical (up to f32 rounding) to the
    factorization of the reference's normalized 5x5 kernel."""
    sigma = 1.0 / 2.35482
    d = np.arange(-PAD, PAD + 1, dtype=np.float64)
    w = np.exp(-(d * d) / (2.0 * sigma * sigma))
    return (w / w.sum()).astype(np.float32)


def _banded(taps: np.ndarray) -> np.ndarray:
    """B[pi, po] = taps[pi - po]: matmul(lhsT=B[:cin,:cout], rhs=x) gives
    t[po, :] = sum_d taps[d] * x[po + d, :] (valid Y correlation)."""
    Bm = np.zeros((128, CHUNK), np.float32)
    for po in range(CHUNK):
        Bm[po : po + 2 * PAD + 1, po] = taps
    return Bm


def _row_chunks():
    """(r0, cin, cout) covering all 768 output rows of one padded view."""
    chunks = []
    r0 = 0
    while r0 < H:
        cout = min(CHUNK, H - r0)
        chunks.append((r0, cout + 2 * PAD, cout))
        r0 += cout
    return chunks


X_STRIPES = [(0, 512), (512, 512), (1024, WT - 1024)]


def _build_v2(with_pm2: bool):
    """v2: PE does Y (fp32, exact) [+ X +-2 taps in bf16]; ACT does the X
    center tap; DVE does the X +-1 taps; gpsimd pre-sums the +-2 operand."""
    f32 = mybir.dt.float32
    bf16 = mybir.dt.bfloat16
    wx = _taps()
    nc = bacc.Bacc("TRN2", target_bir_lowering=False, debug=False)
    xp = nc.dram_tensor("xp", [B_LOC, HP, WP], f32, kind="ExternalInput")
    bY = nc.dram_tensor("bY", [128, CHUNK], f32, kind="ExternalInput")
    bB = nc.dram_tensor("bB", [128, CHUNK], bf16, kind="ExternalInput")
    y = nc.dram_tensor("y", [B_LOC, H, W], f32, kind="ExternalOutput")

    with tile.TileContext(nc) as tc:
        with (
            tc.tile_pool(name="const", bufs=1) as cpool,
            tc.tile_pool(name="xin", bufs=4) as inpool,
            tc.tile_pool(name="ubf", bufs=3) as upool,
            tc.tile_pool(name="ps", bufs=2, space="PSUM") as pspool,
            tc.tile_pool(name="xout", bufs=4) as outpool,
        ):
            bt = cpool.tile([128, CHUNK], f32)
            nc.sync.dma_start(bt[:], bY[:])
            if with_pm2:
                bb = cpool.tile([128, CHUNK], bf16)
                nc.sync.dma_start(bb[:], bB[:])
            for img in range(B_LOC):
                for r0, cin, cout in _row_chunks():
                    xin = inpool.tile([128, WP], f32, tag="xin")
                    nc.sync.dma_start(xin[:cin, :], xp[img, r0 : r0 + cin, :])
                    if with_pm2:
                        ubf = upool.tile([128, WT], bf16, tag="ubf")
                        nc.gpsimd.tensor_tensor(
                            ubf[:cin, :],
                            xin[:cin, 0:WT],
                            xin[:cin, 4 : 4 + WT],
                            op=mybir.AluOpType.add,
                        )
                    t = pspool.tile([CHUNK, WT], f32, tag="ps")
                    for c0, w in X_STRIPES:
                        nc.tensor.matmul(
                            t[:cout, c0 : c0 + w],
                            bt[:cin, :cout],
                            xin[:cin, c0 + 2 : c0 + 2 + w],
                            start=True,
                            stop=not with_pm2,
                        )
                        if with_pm2:
                            nc.tensor.matmul(
                                t[:cout, c0 : c0 + w],
                                bb[:cin, :cout],
                                ubf[:cin, c0 : c0 + w],
                                start=False,
                                stop=True,
                            )
                    out = outpool.tile([CHUNK, W], f32, tag="xout")
                    nc.scalar.activation(
                        out[:cout, :],
                        t[:cout, 2 : 2 + W],
                        mybir.ActivationFunctionType.Copy,
                        scale=float(wx[2]),
                    )
                    for d in (1, 3):
                        nc.vector.scalar_tensor_tensor(
                            out[:cout, :],
                            t[:cout, d : d + W],
                            float(wx[1]),
                            out[:cout, :],
                            op0=mybir.AluOpType.mult,
                            op1=mybir.AluOpType.add,
                        )
                    nc.sync.dma_start(y[img, r0 : r0 + cout, :], out[:cout, :])
    nc.finalize()
    return nc


def _build_v1():
    """v1 baseline: Y via fp32 banded matmul, X all 5 taps on ACT+DVE."""
    f32 = mybir.dt.float32
    wx = _taps()
    nc = bacc.Bacc("TRN2", target_bir_lowering=False, debug=False)
    xp = nc.dram_tensor("xp", [B_LOC, HP, WP], f32, kind="ExternalInput")
    bY = nc.dram_tensor("bY", [128, CHUNK], f32, kind="ExternalInput")
    nc.dram_tensor("bB", [128, CHUNK], mybir.dt.bfloat16, kind="ExternalInput")
    y = nc.dram_tensor("y", [B_LOC, H, W], f32, kind="ExternalOutput")

    with tile.TileContext(nc) as tc:
        with (
            tc.tile_pool(name="const", bufs=1) as cpool,
            tc.tile_pool(name="xin", bufs=4) as inpool,
            tc.tile_pool(name="ps", bufs=2, space="PSUM") as pspool,
            tc.tile_pool(name="xout", bufs=4) as outpool,
        ):
            bt = cpool.tile([128, CHUNK], f32)
            nc.sync.dma_start(bt[:], bY[:])
            for img in range(B_LOC):
                for r0, cin, cout in _row_chunks():
                    xin = inpool.tile([128, WP], f32, tag="xin")
                    nc.sync.dma_start(xin[:cin, :], xp[img, r0 : r0 + cin, :])
                    t = pspool.tile([CHUNK, WT], f32, tag="ps")
                    for c0, w in X_STRIPES:
                        nc.tensor.matmul(
                            t[:cout, c0 : c0 + w],
                            bt[:cin, :cout],
                            xin[:cin, c0 + 2 : c0 + 2 + w],
                            start=True,
                            stop=True,
                        )
                    out = outpool.tile([CHUNK, W], f32, tag="xout")
                    nc.scalar.activation(
                        out[:cout, :],
                        t[:cout, 2 : 2 + W],
                        mybir.ActivationFunctionType.Copy,
                        scale=float(wx[2]),
                    )
                    for d in (0, 1, 3, 4):
                        nc.vector.scalar_tensor_tensor(
                            out[:cout, :],
                            t[:cout, d : d + W],
                            float(wx[d]),
                            out[:cout, :],
                            op0=mybir.AluOpType.mult,
                            op1=mybir.AluOpType.add,
                        )
                    nc.sync.dma_start(y[img, r0 : r0 + cout, :], out[:cout, :])
    nc.finalize()
    return nc


_CACHE: dict = {}


def _get_program(mode: str):
    if mode not in _CACHE:
        if mode == "v1":
            _CACHE[mode] = _build_v1()
        elif mode == "d":
            _CACHE[mode] = _build_v2(with_pm2=False)
        elif mode == "v2":
            _CACHE[mode] = _build_v2(with_pm2=True)
        else:
            raise ValueError(mode)
    return _CACHE[mode]


def _run(x, trace: bool = False, mode: str = MODE, **spmd_kwargs):
    x = np.ascontiguousarray(np.asarray(x, dtype=np.float32))
    assert x.shape == (B_FULL, H, W), x.shape
    xq = np.pad(x, ((0, 0), (PAD, PAD), (0, 0)), mode="reflect")
    xq = np.pad(xq, ((0, 0), (0, 0), (PADX, PADX)), mode="wrap")
    taps = _taps()
    Bm = _banded(taps)
    Bb = (Bm * (taps[0] / taps[2])).astype(ml_dtypes.bfloat16)
    in_maps = [
        {
            "xp": np.ascontiguousarray(xq[i * B_LOC : (i + 1) * B_LOC]),
            "bY": Bm,
            "bB": Bb,
        }
        for i in range(N_CORES)
    ]
    nc = _get_program(mode)
    res = run_bass_kernel_spmd(
        nc, in_maps, list(range(N_CORES)), trace=trace, **spmd_kwargs
    )
    out = np.concatenate([r["y"] for r in res.results], axis=0)
    return np.ascontiguousarray(out.astype(np.float32, copy=False)), res


def kernel(x):
    out, _ = _run(x)
    return out
